# revision 17
# baseline (speedup 1.0000x reference)
"""Trainium2 Bass kernel for nn_Attention_3633542332637 (linear/cosine attention).

Math (per batch n):
  q = x @ Wq.T ; k = x @ Wk.T ; v = x @ Wv.T          (S=4096, D=1024, H=16, HD=64)
  q,k L2-normalized per head over HD; k,v masked; v /= mask.sum()**sigmoid(nc)
  kv_h = k_h^T @ v_h  (64x64) ; attn_h = q_h @ kv_h ; out = attn @ Wo.T

Sharding: core c = 2n + j handles batch n = c//2. k/v are projected over the
FULL sequence but only for this core's 8 heads (j picks heads 8j..8j+8 via a
host-sliced half of Wk/Wv), giving a complete per-head kv with no reduction.
The pair then exchanges kv via a pairwise AllGather (64 KB), which overlaps
with the q projection. q/attn/out run over the local sequence half only (the
host permutes x.T to [partner half | own half] so the SPMD program always
treats chunks 4..7 as local).

All matmul operands are bf16 (full PE rate incl. the small-N kv matmuls,
which fp32r runs at 1/4 rate below 256 columns); accumulation is fp32 in PSUM
and all normalization statistics are fp32. q-hat stays in SBUF (bf16) — no
DRAM spill. The gathered kv is laid out as 8 block-diagonal [128,128] tiles
(one per head PAIR), so attn is 32 full-width matmuls. mask / v-denominator
fold into one [t, h] broadcast multiply on k (host-prepped); the q-head-norm
uses a block-diagonal ones matmul (sel) to reduce over partitions.
"""

import numpy as np
import ml_dtypes

import concourse.bass as bass
import concourse.mybir as mybir
import concourse.tile as tile
from concourse import bacc
from concourse.bass_utils import run_bass_kernel_spmd

N, S, D = 4, 4096, 1024
H, HD = 16, 64
P = 128
DC = D // P            # 8 contraction chunks
SLOC = S // 2          # 2048 local positions
HLOC = H // 2          # 8 heads per core
DH = HLOC * HD         # 512 = head-half feature width
NT = S // P            # 32 token tiles (full sequence)
NCHF = S // 512        # 8 full-sequence 512-token chunks
NCH = SLOC // 512      # 4 local chunks
NCORES = 8

F32 = mybir.dt.float32
F32R = mybir.dt.float32r
BF16 = mybir.dt.bfloat16
SQUARE = mybir.ActivationFunctionType.Square

REPLICA_GROUPS = [[0, 1], [2, 3], [4, 5], [6, 7]]

_BUILD_CACHE = {}


def build(reps=1, phases="both", cc=True, dbg=False, kvdt=BF16):
    key = ("nc", reps, phases, cc, dbg, str(kvdt))
    if key in _BUILD_CACHE:
        return _BUILD_CACHE[key]
    nc = bacc.Bacc("TRN2", target_bir_lowering=False, debug=False)

    # ---- I/O ----
    # x[n].T (bf16) with columns permuted to [partner half | own half]
    xt = nc.declare_dram_parameter("xt", [D, S], BF16, isOutput=False)
    # head-half slices of Wk.T / Wv.T for this core's 8 heads
    wk = nc.declare_dram_parameter("wk", [D, DH], BF16, isOutput=False)
    wv = nc.declare_dram_parameter("wv", [D, DH], BF16, isOutput=False)
    wq = nc.declare_dram_parameter("wq", [D, D], BF16, isOutput=False)
    wo = nc.declare_dram_parameter("wo", [D, D], BF16, isOutput=False)
    # mvs[p, tt, h] = mask_perm[128*tt + p] * vscale[h0 + h]  (this core's heads)
    mvs = nc.declare_dram_parameter("mvs", [P, NT, HLOC], F32, isOutput=False)
    sel = nc.declare_dram_parameter("sel", [P, P], F32, isOutput=False)  # block-diag ones
    out = nc.declare_dram_parameter("out", [SLOC, D], F32, isOutput=True)
    if dbg:
        dbg_kvblk = nc.declare_dram_parameter("dbg_kvblk", [P, DC, P], BF16, isOutput=True)
        dbg_qh = nc.declare_dram_parameter("dbg_qh", [P, DC, SLOC], BF16, isOutput=True)
        dbg_at = nc.declare_dram_parameter("dbg_at", [P, NCH, DC, 512], BF16, isOutput=True)
        dbg_khat = nc.declare_dram_parameter("dbg_khat", [P, HLOC, HD], BF16, isOutput=True)
        dbg_v = nc.declare_dram_parameter("dbg_v", [P, DH], BF16, isOutput=True)
        dbg_kvraw = nc.declare_dram_parameter("dbg_kvraw", [64, DH], BF16, isOutput=True)

    def dram3(t):
        return t.ap().rearrange("(dc p) x -> p dc x", p=P)

    with tile.TileContext(nc) as tc:
        with (
            tc.tile_pool(name="consts", bufs=1) as consts,
            tc.tile_pool(name="cdram", bufs=2, space="DRAM") as cdram,
        ):
            wk_sb = consts.tile([P, DC, DH], BF16)
            wv_sb = consts.tile([P, DC, DH], BF16)
            sel_sb = consts.tile([P, P], F32R)
            mvs_sb = consts.tile([P, NT, HLOC], F32)
            xloc = consts.tile([P, DC, SLOC], BF16)       # local x half
            wq_sb = consts.tile([P, DC, D], BF16)
            wo_sb = consts.tile([P, DC, D], BF16)
            qh_sb = consts.tile([P, DC, SLOC], BF16)      # q-hat, [e-rows, tokens]
            # block-diag kv per head pair: kvblk[:, et] = diag(kv_2et, kv_2et+1)
            kvblk = consts.tile([P, DC, P], BF16)
            at_sb = consts.tile([P, NCH, DC, 512], BF16)  # attn out, [e-rows, tokens]

            nc.sync.dma_start(out=sel_sb[:], in_=sel.ap().bitcast(F32R))
            nc.sync.dma_start(out=mvs_sb[:], in_=mvs.ap())
            nc.sync.dma_start(out=wk_sb[:], in_=dram3(wk))
            nc.sync.dma_start(out=wv_sb[:], in_=dram3(wv))

            xt3 = dram3(xt)

            cc_in = cdram.tile([64, DH], BF16, tag="cc_in")
            cc_out = cdram.tile([P, DH], BF16, tag="cc_out")

            # off-diagonal blocks of kvblk stay zero across reps
            nc.vector.memset(kvblk[:].bitcast(F32), 0.0)

            def emit_phase1():
              with (
                tc.tile_pool(name="pAx", bufs=2) as pAx,
                tc.tile_pool(name="pAwork", bufs=3) as pAwork,
                tc.tile_pool(name="pAstats", bufs=4) as pAstats,
                tc.tile_pool(name="pApsum", bufs=2, space="PSUM") as pApsum,
                tc.tile_pool(name="kvpool", bufs=1, space="PSUM") as kvpool,
              ):
                kv_ps = kvpool.tile([64, DH], F32)  # 1 bank, accumulated all phase

                def emit_kv(khat, v_sb, tt):
                    # NOTE: start=True clears has_written bits for the WHOLE
                    # bank, so only the very first matmul of the bank may set it.
                    for hh in range(HLOC):
                        nc.tensor.matmul(
                            kv_ps[:, HD * hh : HD * (hh + 1)],
                            lhsT=khat[:, hh, :],
                            rhs=v_sb[:, HD * hh : HD * (hh + 1)],
                            start=(tt == 0 and hh == 0),
                            stop=(tt == NT - 1 and hh == HLOC - 1),
                            skip_group_check=True,
                        )

                pend = None
                for c in range(NCHF):
                    if c < NCH:
                        x_sb = pAx.tile([P, DC, 512], BF16)
                        nc.sync.dma_start(
                            out=x_sb[:], in_=xt3[:, :, 512 * c : 512 * (c + 1)]
                        )
                    elif c == NCH:
                        # local half: one resident DMA, reused in phases 2/3
                        nc.sync.dma_start(out=xloc[:], in_=xt3[:, :, SLOC:])
                    for tt4 in range(4):
                        tt = 4 * c + tt4
                        if c < NCH:
                            xs = x_sb[:, :, 128 * tt4 : 128 * (tt4 + 1)]
                        else:
                            t0 = 512 * (c - NCH) + 128 * tt4
                            xs = xloc[:, :, t0 : t0 + 128]
                        kps = pApsum.tile([P, DH], F32, tag="kps")
                        for dc in range(DC):
                            nc.tensor.matmul(
                                kps[:],
                                lhsT=xs[:, dc, :],
                                rhs=wk_sb[:, dc, :],
                                start=(dc == 0),
                                stop=(dc == DC - 1),
                            )
                        vps = pApsum.tile([P, DH], F32, tag="vps")
                        for dc in range(DC):
                            nc.tensor.matmul(
                                vps[:],
                                lhsT=xs[:, dc, :],
                                rhs=wv_sb[:, dc, :],
                                start=(dc == 0),
                                stop=(dc == DC - 1),
                            )
                        # k normalization factors (fp32 stats)
                        ksqf = pAstats.tile([P, DH], F32, tag="ksqf")
                        nc.scalar.activation(out=ksqf[:], in_=kps[:], func=SQUARE)
                        ksq = pAstats.tile([P, HLOC], F32, tag="ksq")
                        nc.vector.reduce_sum(
                            out=ksq[:],
                            in_=ksqf[:].rearrange("p (h a) -> p h a", h=HLOC),
                            axis=mybir.AxisListType.X,
                        )
                        r = pAstats.tile([P, HLOC], F32, tag="r")
                        nc.scalar.sqrt(out=r[:], in_=ksq[:])
                        nc.vector.tensor_scalar_max(out=r[:], in0=r[:], scalar1=1e-12)
                        nc.vector.reciprocal(out=r[:], in_=r[:])
                        nc.vector.tensor_mul(
                            out=r[:], in0=r[:], in1=mvs_sb[:, tt, :]
                        )
                        khat = pAwork.tile([P, HLOC, HD], kvdt, tag="khat")
                        nc.vector.tensor_tensor(
                            khat[:],
                            kps[:].rearrange("p (h a) -> p h a", h=HLOC),
                            r[:, :, None].to_broadcast((P, HLOC, HD)),
                            mybir.AluOpType.mult,
                        )
                        v_sb = pAwork.tile([P, DH], kvdt, tag="v_sb")
                        nc.scalar.copy(out=v_sb[:], in_=vps[:])
                        # software-pipelined: kv matmuls for the PREVIOUS tile,
                        # so PE never waits on this tile's stats chain
                        if pend is not None:
                            emit_kv(*pend)
                        pend = (khat, v_sb, tt)
                emit_kv(*pend)
                if dbg:
                    nc.sync.dma_start(out=dbg_khat.ap(), in_=pend[0][:])
                    nc.sync.dma_start(out=dbg_v.ap(), in_=pend[1][:])

                # kv -> DRAM bounce -> pairwise AllGather -> kvblk diagonals
                kv_sb = pAwork.tile([64, DH], BF16, tag="kv_sb")
                nc.vector.tensor_copy(out=kv_sb[:], in_=kv_ps[:])
                if dbg:
                    nc.sync.dma_start(out=dbg_kvraw.ap(), in_=kv_sb[:])
                nc.gpsimd.dma_start(out=cc_in[:], in_=kv_sb[:])
                if cc:
                    nc.gpsimd.collective_compute(
                        "AllGather",
                        mybir.AluOpType.bypass,
                        replica_groups=REPLICA_GROUPS,
                        ins=[cc_in.opt()],
                        outs=[cc_out.opt()],
                    )
                    # head h = 8g + 2i + two lands in kvblk[:, et=4g+i]
                    v = cc_out[:].rearrange(
                        "(g q) (i two e) -> g q i two e", g=2, i=4, two=2
                    )
                    nc.gpsimd.dma_start(out=kvblk[0:64, 0:4, 0:64], in_=v[0, :, :, 0, :])
                    nc.gpsimd.dma_start(out=kvblk[0:64, 4:8, 0:64], in_=v[1, :, :, 0, :])
                    nc.gpsimd.dma_start(out=kvblk[64:P, 0:4, 64:P], in_=v[0, :, :, 1, :])
                    nc.gpsimd.dma_start(out=kvblk[64:P, 4:8, 64:P], in_=v[1, :, :, 1, :])
                else:  # timing-ablation only: duplicate own kv into both halves
                    v2 = cc_in[:].rearrange("q (i two e) -> q i two e", i=4, two=2)
                    nc.gpsimd.dma_start(out=kvblk[0:64, 0:4, 0:64], in_=v2[:, :, 0, :])
                    nc.gpsimd.dma_start(out=kvblk[0:64, 4:8, 0:64], in_=v2[:, :, 0, :])
                    nc.gpsimd.dma_start(out=kvblk[64:P, 0:4, 64:P], in_=v2[:, :, 1, :])
                    nc.gpsimd.dma_start(out=kvblk[64:P, 4:8, 64:P], in_=v2[:, :, 1, :])

            def emit_phase2():
              with (
                tc.tile_pool(name="pQwork", bufs=3) as pQwork,
                tc.tile_pool(name="pQpsum", bufs=3, space="PSUM") as pQpsum,
                tc.tile_pool(name="pNpsum", bufs=2, space="PSUM") as pNpsum,
              ):
                def emit_norm(qps, q2, cl, et):
                    nps = pNpsum.tile([P, 512], F32, tag="nps")
                    nc.tensor.matmul(nps[:], lhsT=sel_sb[:], rhs=q2[:],
                                     start=True, stop=True)
                    rn = pQwork.tile([P, 512], F32, tag="rn")
                    nc.scalar.sqrt(out=rn[:], in_=nps[:])
                    nc.vector.tensor_scalar_max(out=rn[:], in0=rn[:], scalar1=1e-12)
                    nc.vector.reciprocal(out=rn[:], in_=rn[:])
                    nc.vector.tensor_mul(
                        out=qh_sb[:, et, 512 * cl : 512 * (cl + 1)],
                        in0=qps[:], in1=rn[:],
                    )

                pend = None
                for cl in range(NCH):
                    for et in range(DC):
                        qps = pQpsum.tile([P, 512], F32, tag="qps")
                        for dc in range(DC):
                            nc.tensor.matmul(
                                qps[:],
                                lhsT=wq_sb[:, dc, 128 * et : 128 * (et + 1)],
                                rhs=xloc[:, dc, 512 * cl : 512 * (cl + 1)],
                                start=(dc == 0),
                                stop=(dc == DC - 1),
                            )
                        q2 = pQwork.tile([P, 512], F32R, tag="q2")
                        nc.scalar.activation(out=q2[:], in_=qps[:], func=SQUARE)
                        if pend is not None:
                            emit_norm(*pend)
                        pend = (qps, q2, cl, et)
                emit_norm(*pend)

            def emit_phase3():
              with (
                tc.tile_pool(name="pOwork", bufs=3) as pOwork,
                tc.tile_pool(name="pApsum3", bufs=3, space="PSUM") as pApsum3,
                tc.tile_pool(name="pOpsum", bufs=2, space="PSUM") as pOpsum,
              ):
                for cl in range(NCH):
                    for et in range(DC):
                        aps = pApsum3.tile([P, 512], F32, tag="aps")
                        nc.tensor.matmul(
                            aps[:],
                            lhsT=kvblk[:, et, :],
                            rhs=qh_sb[:, et, 512 * cl : 512 * (cl + 1)],
                            start=True, stop=True,
                        )
                        nc.scalar.copy(out=at_sb[:, cl, et, :], in_=aps[:])
                for cl in range(NCH):
                    for tt4 in range(4):
                        o_sb = pOwork.tile([P, D], F32, tag="o_sb")
                        for half in range(2):
                            ops = pOpsum.tile([P, 512], F32, tag="ops")
                            for ec in range(DC):
                                nc.tensor.matmul(
                                    ops[:],
                                    lhsT=at_sb[:, cl, ec, 128 * tt4 : 128 * (tt4 + 1)],
                                    rhs=wo_sb[:, ec, 512 * half : 512 * (half + 1)],
                                    start=(ec == 0),
                                    stop=(ec == DC - 1),
                                )
                            nc.scalar.copy(
                                out=o_sb[:, 512 * half : 512 * (half + 1)], in_=ops[:]
                            )
                        t0 = 512 * cl + 128 * tt4
                        nc.sync.dma_start(out=out.ap()[t0 : t0 + P, :], in_=o_sb[:])

            if phases in ("p23", "none") or reps == 0:
                nc.vector.memset(qh_sb[:].bitcast(F32), 0.0)
                nc.vector.memset(at_sb[:].bitcast(F32), 0.0)
            for _rep in range(reps):
                if phases in ("both", "p1"):
                    emit_phase1()
                if phases in ("both", "p23"):
                    # prefetch wq/wo late so they don't delay phase-1 x DMAs
                    nc.sync.dma_start(out=wq_sb[:], in_=dram3(wq))
                    nc.sync.dma_start(out=wo_sb[:], in_=dram3(wo))
                    emit_phase2()
                    emit_phase3()
            if dbg:
                nc.sync.dma_start(out=dbg_kvblk.ap(), in_=kvblk[:])
                nc.sync.dma_start(out=dbg_qh.ap(), in_=qh_sb[:])
                nc.sync.dma_start(out=dbg_at.ap(), in_=at_sb[:])

    nc.finalize()
    _BUILD_CACHE[key] = nc
    return nc


def _sel_np():
    e = np.arange(P)
    return (e[:, None] // HD == e[None, :] // HD).astype(np.float32)


def make_in_maps(x, mask, Wq, Wk, Wv, Wo, norm_const):
    x = np.asarray(x)
    mask = np.asarray(mask)
    Wq = np.asarray(Wq); Wk = np.asarray(Wk); Wv = np.asarray(Wv); Wo = np.asarray(Wo)
    norm_const = np.asarray(norm_const)

    bf = ml_dtypes.bfloat16
    wkT = np.ascontiguousarray(Wk.T)
    wvT = np.ascontiguousarray(Wv.T)
    wqT = np.ascontiguousarray(Wq.T).astype(bf)
    woT = np.ascontiguousarray(Wo.T).astype(bf)
    sel = _sel_np()

    m32 = mask.astype(np.float32)
    # denom[n, h] = mask[n].sum() ** sigmoid(norm_const[h]); vscale = 1/denom
    sig = 1.0 / (1.0 + np.exp(-norm_const.astype(np.float32).reshape(H)))
    msum = m32.sum(axis=1)  # [N]
    denom = msum[:, None] ** sig[None, :]  # [N, H]
    vscale = (1.0 / denom).astype(np.float32)

    in_maps = []
    xts = {n: np.ascontiguousarray(x[n].T) for n in range(N)}
    for c in range(NCORES):
        n, j = c // 2, c % 2
        other = 1 - j
        # permuted x.T: [partner half | own half]
        xp = np.concatenate(
            [xts[n][:, other * SLOC : (other + 1) * SLOC],
             xts[n][:, j * SLOC : (j + 1) * SLOC]], axis=1
        ).astype(bf)
        mp = np.concatenate(
            [m32[n, other * SLOC : (other + 1) * SLOC],
             m32[n, j * SLOC : (j + 1) * SLOC]]
        )
        mcol = mp.reshape(NT, P).T  # [p, tt]
        mvs = np.ascontiguousarray(
            mcol[:, :, None] * vscale[n][None, None, 8 * j : 8 * j + HLOC]
        ).astype(np.float32)
        in_maps.append({
            "xt": np.ascontiguousarray(xp),
            "wk": np.ascontiguousarray(wkT[:, DH * j : DH * (j + 1)]).astype(bf),
            "wv": np.ascontiguousarray(wvT[:, DH * j : DH * (j + 1)]).astype(bf),
            "wq": wqT, "wo": woT,
            "mvs": mvs, "sel": sel,
        })
    return in_maps


def kernel(x, mask, Wq, Wk, Wv, Wo, norm_const):
    in_maps = make_in_maps(x, mask, Wq, Wk, Wv, Wo, norm_const)
    nc = build()
    res = run_bass_kernel_spmd(nc, in_maps, core_ids=list(range(NCORES)))
    out = np.empty((N, S, D), dtype=np.float32)
    for c in range(NCORES):
        n, j = c // 2, c % 2
        out[n, j * SLOC : (j + 1) * SLOC, :] = res.results[c]["out"]
    return out


# revision 25
# speedup vs baseline: 1.3472x; 1.3472x over previous
"""Trainium2 Bass kernel for nn_Attention_3633542332637 (linear/cosine attention).

Math (per batch n):
  q = x @ Wq.T ; k = x @ Wk.T ; v = x @ Wv.T          (S=4096, D=1024, H=16, HD=64)
  q,k L2-normalized per head over HD; k,v masked; v /= mask.sum()**sigmoid(nc)
  kv_h = k_h^T @ v_h  (64x64) ; attn_h = q_h @ kv_h ; out = attn @ Wo.T

Sharding: core c = 2n + j handles batch n = c//2. k/v are projected over the
FULL sequence but only for this core's 8 heads (j picks heads 8j..8j+8 via a
host-sliced half of Wk/Wv), giving a complete per-head kv with no reduction.
The pair then exchanges kv via a pairwise AllGather (64 KB), which overlaps
with the q projection. q/attn/out run over the local sequence half only (the
host permutes x.T to [partner half | own half] so the SPMD program always
treats chunks 4..7 as local).

All matmul operands are bf16 (full PE rate incl. the small-N kv matmuls,
which fp32r runs at 1/4 rate below 256 columns); accumulation is fp32 in PSUM
and all normalization statistics are fp32. q-hat stays in SBUF (bf16) — no
DRAM spill. The gathered kv is laid out as 8 block-diagonal [128,128] tiles
(one per head PAIR), so attn is 32 full-width matmuls. mask / v-denominator
fold into one [t, h] broadcast multiply on k (host-prepped); the q-head-norm
uses a block-diagonal ones matmul (sel) to reduce over partitions.
"""

import numpy as np
import ml_dtypes

import concourse.bass as bass
import concourse.mybir as mybir
import concourse.tile as tile
from concourse import bacc
from concourse.bass_utils import run_bass_kernel_spmd

N, S, D = 4, 4096, 1024
H, HD = 16, 64
P = 128
DC = D // P            # 8 contraction chunks
SLOC = S // 2          # 2048 local positions
HLOC = H // 2          # 8 heads per core
DH = HLOC * HD         # 512 = head-half feature width
NT = S // P            # 32 token tiles (full sequence)
NCHF = S // 512        # 8 full-sequence 512-token chunks
NCH = SLOC // 512      # 4 local chunks
NCORES = 8

F32 = mybir.dt.float32
F32R = mybir.dt.float32r
BF16 = mybir.dt.bfloat16
SQUARE = mybir.ActivationFunctionType.Square

REPLICA_GROUPS = [[0, 1], [2, 3], [4, 5], [6, 7]]

_BUILD_CACHE = {}


def build(reps=1, phases="both", cc=True, dbg=False, kvdt=BF16):
    key = ("nc", reps, phases, cc, dbg, str(kvdt))
    if key in _BUILD_CACHE:
        return _BUILD_CACHE[key]
    nc = bacc.Bacc("TRN2", target_bir_lowering=False, debug=False)

    # ---- I/O ----
    # x[n].T (bf16) with columns permuted to [partner half | own half]
    xt = nc.declare_dram_parameter("xt", [D, S], BF16, isOutput=False)
    # head-half slices of Wk.T / Wv.T for this core's 8 heads
    wk = nc.declare_dram_parameter("wk", [D, DH], BF16, isOutput=False)
    wv = nc.declare_dram_parameter("wv", [D, DH], BF16, isOutput=False)
    wq = nc.declare_dram_parameter("wq", [D, D], BF16, isOutput=False)
    wo = nc.declare_dram_parameter("wo", [D, D], BF16, isOutput=False)
    # mvs[p, tt, h] = mask_perm[128*tt + p] * vscale[h0 + h]  (this core's heads)
    mvs = nc.declare_dram_parameter("mvs", [P, NT, HLOC], F32, isOutput=False)
    sel = nc.declare_dram_parameter("sel", [P, P], F32, isOutput=False)  # block-diag ones
    out = nc.declare_dram_parameter("out", [SLOC, D], F32, isOutput=True)
    if dbg:
        dbg_kvblk = nc.declare_dram_parameter("dbg_kvblk", [P, DC, P], BF16, isOutput=True)
        dbg_qh = nc.declare_dram_parameter("dbg_qh", [P, DC, SLOC], BF16, isOutput=True)
        dbg_at = nc.declare_dram_parameter("dbg_at", [P, NCH, DC, 512], BF16, isOutput=True)
        dbg_khat = nc.declare_dram_parameter("dbg_khat", [P, HLOC, HD], BF16, isOutput=True)
        dbg_v = nc.declare_dram_parameter("dbg_v", [P, DH], BF16, isOutput=True)
        dbg_kvraw = nc.declare_dram_parameter("dbg_kvraw", [64, DH], BF16, isOutput=True)

    def dram3(t):
        return t.ap().rearrange("(dc p) x -> p dc x", p=P)

    with tile.TileContext(nc) as tc:
        with (
            tc.tile_pool(name="consts", bufs=1) as consts,
            tc.tile_pool(name="cdram", bufs=2, space="DRAM") as cdram,
        ):
            wk_sb = consts.tile([P, DC, DH], BF16)
            wv_sb = consts.tile([P, DC, DH], BF16)
            sel_sb = consts.tile([P, P], F32R)
            mvs_sb = consts.tile([P, NT, HLOC], F32)
            xloc = consts.tile([P, DC, SLOC], BF16)       # local x half
            wq_sb = consts.tile([P, DC, D], BF16)
            wo_sb = consts.tile([P, DC, D], BF16)
            qh_sb = consts.tile([P, DC, SLOC], BF16)      # q-hat, [e-rows, tokens]
            # block-diag kv per head pair: kvblk[:, et] = diag(kv_2et, kv_2et+1)
            kvblk = consts.tile([P, DC, P], BF16)
            at_sb = consts.tile([P, NCH, DC, 512], BF16)  # attn out, [e-rows, tokens]

            nc.sync.dma_start(out=wk_sb[:], in_=dram3(wk))
            nc.sync.dma_start(out=sel_sb[:], in_=sel.ap().bitcast(F32R))

            xt3 = dram3(xt)

            ccA_in = cdram.tile([64, DH], F32, tag="ccA_in")
            ccA_out = cdram.tile([P, DH], F32, tag="ccA_out")
            ccB_in = cdram.tile([64, DH], F32, tag="ccB_in")
            ccB_out = cdram.tile([P, DH], F32, tag="ccB_out")

            # off-diagonal blocks of kvblk stay zero across reps
            nc.vector.memset(kvblk[:].bitcast(F32), 0.0)

            def emit_phase1(first_rep=True):
              with (
                tc.tile_pool(name="pAx", bufs=2) as pAx,
                tc.tile_pool(name="pAwork", bufs=3) as pAwork,
                tc.tile_pool(name="pAstats", bufs=4) as pAstats,
                tc.tile_pool(name="pApsum", bufs=2, space="PSUM") as pApsum,
                tc.tile_pool(name="kvpool", bufs=1, space="PSUM") as kvpool,
              ):
                # two half-sequence kv accumulators so each half's exchange
                # overlaps the remaining compute (CC#1 under tiles 16-31,
                # CC#2 under the q projection)
                NTH = NT // 2
                kv_psA = kvpool.tile([64, DH], F32, tag="kvA")
                kv_psB = kvpool.tile([64, DH], F32, tag="kvB")

                def emit_kv(khat, v_sb, tt):
                    kv_ps = kv_psA if tt < NTH else kv_psB
                    t0 = tt % NTH
                    # NOTE: start=True clears has_written bits for the WHOLE
                    # bank, so only the very first matmul of the bank may set it.
                    for hh in range(HLOC):
                        nc.tensor.matmul(
                            kv_ps[:, HD * hh : HD * (hh + 1)],
                            lhsT=khat[:, hh, :],
                            rhs=v_sb[:, HD * hh : HD * (hh + 1)],
                            start=(t0 == 0 and hh == 0),
                            stop=(t0 == NTH - 1 and hh == HLOC - 1),
                            skip_group_check=True,
                        )

                def emit_exchange(kv_ps, c_in, c_out, tag):
                    kvh = pAwork.tile([64, DH], F32, tag=f"kvh{tag}")
                    nc.vector.tensor_copy(out=kvh[:], in_=kv_ps[:])
                    nc.gpsimd.dma_start(out=c_in[:], in_=kvh[:])
                    if cc:
                        nc.gpsimd.collective_compute(
                            "AllGather",
                            mybir.AluOpType.bypass,
                            replica_groups=REPLICA_GROUPS,
                            ins=[c_in.opt()],
                            outs=[c_out.opt()],
                        )
                    else:  # timing ablation: fake the gather with local copies
                        nc.gpsimd.dma_start(out=c_out[0:64, :], in_=c_in[:])
                        nc.gpsimd.dma_start(out=c_out[64:P, :], in_=c_in[:])

                pend = None
                for c in range(NCHF):
                    if c < NCH:
                        x_sb = pAx.tile([P, DC, 512], BF16)
                        if c == 0:
                            # split first chunk so tile 0 can start ASAP;
                            # interleave wv/mvs loads between the quarters
                            nc.sync.dma_start(
                                out=x_sb[:, :, 0:128], in_=xt3[:, :, 0:128]
                            )
                            if first_rep:
                                nc.sync.dma_start(out=wv_sb[:], in_=dram3(wv))
                            nc.sync.dma_start(
                                out=x_sb[:, :, 128:256], in_=xt3[:, :, 128:256]
                            )
                            if first_rep:
                                nc.sync.dma_start(out=mvs_sb[:], in_=mvs.ap())
                            nc.sync.dma_start(
                                out=x_sb[:, :, 256:512], in_=xt3[:, :, 256:512]
                            )
                        else:
                            nc.sync.dma_start(
                                out=x_sb[:], in_=xt3[:, :, 512 * c : 512 * (c + 1)]
                            )
                    elif c == NCH:
                        # local half: one resident DMA, reused in phases 2/3
                        nc.sync.dma_start(out=xloc[:], in_=xt3[:, :, SLOC:])
                    for tt4 in range(4):
                        tt = 4 * c + tt4
                        if c < NCH:
                            xs = x_sb[:, :, 128 * tt4 : 128 * (tt4 + 1)]
                        else:
                            t0 = 512 * (c - NCH) + 128 * tt4
                            xs = xloc[:, :, t0 : t0 + 128]
                        kps = pApsum.tile([P, DH], F32, tag="kps")
                        for dc in range(DC):
                            nc.tensor.matmul(
                                kps[:],
                                lhsT=xs[:, dc, :],
                                rhs=wk_sb[:, dc, :],
                                start=(dc == 0),
                                stop=(dc == DC - 1),
                            )
                        vps = pApsum.tile([P, DH], F32, tag="vps")
                        for dc in range(DC):
                            nc.tensor.matmul(
                                vps[:],
                                lhsT=xs[:, dc, :],
                                rhs=wv_sb[:, dc, :],
                                start=(dc == 0),
                                stop=(dc == DC - 1),
                            )
                        # k normalization factors (fp32 stats)
                        ksqf = pAstats.tile([P, DH], F32, tag="ksqf")
                        nc.scalar.activation(out=ksqf[:], in_=kps[:], func=SQUARE)
                        ksq = pAstats.tile([P, HLOC], F32, tag="ksq")
                        nc.vector.reduce_sum(
                            out=ksq[:],
                            in_=ksqf[:].rearrange("p (h a) -> p h a", h=HLOC),
                            axis=mybir.AxisListType.X,
                        )
                        r = pAstats.tile([P, HLOC], F32, tag="r")
                        nc.scalar.sqrt(out=r[:], in_=ksq[:])
                        nc.vector.tensor_scalar_max(out=r[:], in0=r[:], scalar1=1e-12)
                        nc.vector.reciprocal(out=r[:], in_=r[:])
                        nc.vector.tensor_mul(
                            out=r[:], in0=r[:], in1=mvs_sb[:, tt, :]
                        )
                        khat = pAwork.tile([P, HLOC, HD], kvdt, tag="khat")
                        nc.vector.tensor_tensor(
                            khat[:],
                            kps[:].rearrange("p (h a) -> p h a", h=HLOC),
                            r[:, :, None].to_broadcast((P, HLOC, HD)),
                            mybir.AluOpType.mult,
                        )
                        v_sb = pAwork.tile([P, DH], kvdt, tag="v_sb")
                        nc.scalar.copy(out=v_sb[:], in_=vps[:])
                        # software-pipelined: kv matmuls for the PREVIOUS tile,
                        # so PE never waits on this tile's stats chain
                        if pend is not None:
                            emit_kv(*pend)
                            if pend[2] == NTH - 1:
                                emit_exchange(kv_psA, ccA_in, ccA_out, "A")
                        pend = (khat, v_sb, tt)
                emit_kv(*pend)
                if dbg:
                    nc.sync.dma_start(out=dbg_khat.ap(), in_=pend[0][:])
                    nc.sync.dma_start(out=dbg_v.ap(), in_=pend[1][:])
                emit_exchange(kv_psB, ccB_in, ccB_out, "B")

                # gathered halves -> SBUF, sum, scatter into kvblk diagonals.
                # head h = 8g + 2i + two lands in kvblk[:, et=4g+i]
                gA = pAwork.tile([P, DH], F32, tag="gA")
                gB = pAwork.tile([P, DH], F32, tag="gB")
                nc.gpsimd.dma_start(out=gA[:], in_=ccA_out[:])
                nc.gpsimd.dma_start(out=gB[:], in_=ccB_out[:])
                kvsum = pAwork.tile([P, DH], BF16, tag="kvsum")
                nc.vector.tensor_tensor(
                    kvsum[:], gA[:], gB[:], mybir.AluOpType.add
                )
                if dbg:
                    nc.sync.dma_start(out=dbg_kvraw.ap(), in_=kvsum[0:64, :])
                v = kvsum[:].rearrange("p (i two e) -> p i two e", i=4, two=2)
                nc.sync.dma_start(out=kvblk[0:64, 0:4, 0:64], in_=v[0:64, :, 0, :])
                nc.sync.dma_start(out=kvblk[0:64, 4:8, 0:64], in_=v[64:P, :, 0, :])
                nc.sync.dma_start(out=kvblk[64:P, 0:4, 64:P], in_=v[0:64, :, 1, :])
                nc.sync.dma_start(out=kvblk[64:P, 4:8, 64:P], in_=v[64:P, :, 1, :])

            def emit_phase2():
              with (
                tc.tile_pool(name="pQwork", bufs=3) as pQwork,
                tc.tile_pool(name="pQpsum", bufs=3, space="PSUM") as pQpsum,
                tc.tile_pool(name="pNpsum", bufs=2, space="PSUM") as pNpsum,
              ):
                def emit_norm(qps, q2, cl, et):
                    nps = pNpsum.tile([P, 512], F32, tag="nps")
                    nc.tensor.matmul(nps[:], lhsT=sel_sb[:], rhs=q2[:],
                                     start=True, stop=True)
                    rn = pQwork.tile([P, 512], F32, tag="rn")
                    nc.scalar.sqrt(out=rn[:], in_=nps[:])
                    nc.vector.tensor_scalar_max(out=rn[:], in0=rn[:], scalar1=1e-12)
                    nc.vector.reciprocal(out=rn[:], in_=rn[:])
                    nc.vector.tensor_mul(
                        out=qh_sb[:, et, 512 * cl : 512 * (cl + 1)],
                        in0=qps[:], in1=rn[:],
                    )

                pend = None
                for cl in range(NCH):
                    for et in range(DC):
                        qps = pQpsum.tile([P, 512], F32, tag="qps")
                        for dc in range(DC):
                            nc.tensor.matmul(
                                qps[:],
                                lhsT=wq_sb[:, dc, 128 * et : 128 * (et + 1)],
                                rhs=xloc[:, dc, 512 * cl : 512 * (cl + 1)],
                                start=(dc == 0),
                                stop=(dc == DC - 1),
                            )
                        q2 = pQwork.tile([P, 512], F32R, tag="q2")
                        nc.scalar.activation(out=q2[:], in_=qps[:], func=SQUARE)
                        if pend is not None:
                            emit_norm(*pend)
                        pend = (qps, q2, cl, et)
                emit_norm(*pend)

            def emit_phase3():
              with (
                tc.tile_pool(name="pOwork", bufs=3) as pOwork,
                tc.tile_pool(name="pApsum3", bufs=3, space="PSUM") as pApsum3,
                tc.tile_pool(name="pOpsum", bufs=2, space="PSUM") as pOpsum,
              ):
                for cl in range(NCH):
                    for et in range(DC):
                        aps = pApsum3.tile([P, 512], F32, tag="aps")
                        nc.tensor.matmul(
                            aps[:],
                            lhsT=kvblk[:, et, :],
                            rhs=qh_sb[:, et, 512 * cl : 512 * (cl + 1)],
                            start=True, stop=True,
                        )
                        nc.scalar.copy(out=at_sb[:, cl, et, :], in_=aps[:])
                for cl in range(NCH):
                    for tt4 in range(4):
                        o_sb = pOwork.tile([P, D], F32, tag="o_sb")
                        for half in range(2):
                            ops = pOpsum.tile([P, 512], F32, tag="ops")
                            for ec in range(DC):
                                nc.tensor.matmul(
                                    ops[:],
                                    lhsT=at_sb[:, cl, ec, 128 * tt4 : 128 * (tt4 + 1)],
                                    rhs=wo_sb[:, ec, 512 * half : 512 * (half + 1)],
                                    start=(ec == 0),
                                    stop=(ec == DC - 1),
                                )
                            nc.scalar.copy(
                                out=o_sb[:, 512 * half : 512 * (half + 1)], in_=ops[:]
                            )
                        t0 = 512 * cl + 128 * tt4
                        nc.sync.dma_start(out=out.ap()[t0 : t0 + P, :], in_=o_sb[:])

            if phases in ("p23", "none") or reps == 0:
                nc.vector.memset(qh_sb[:].bitcast(F32), 0.0)
                nc.vector.memset(at_sb[:].bitcast(F32), 0.0)
            for _rep in range(reps):
                if phases in ("both", "p1"):
                    emit_phase1(first_rep=(_rep == 0))
                if phases in ("both", "p23"):
                    if _rep == 0:
                        # prefetch wq/wo late so they don't delay phase-1 x DMAs
                        nc.sync.dma_start(out=wq_sb[:], in_=dram3(wq))
                        nc.sync.dma_start(out=wo_sb[:], in_=dram3(wo))
                    emit_phase2()
                    emit_phase3()
            if dbg:
                nc.sync.dma_start(out=dbg_kvblk.ap(), in_=kvblk[:])
                nc.sync.dma_start(out=dbg_qh.ap(), in_=qh_sb[:])
                nc.sync.dma_start(out=dbg_at.ap(), in_=at_sb[:])

    nc.finalize()
    _BUILD_CACHE[key] = nc
    return nc


def _sel_np():
    e = np.arange(P)
    return (e[:, None] // HD == e[None, :] // HD).astype(np.float32)


def make_in_maps(x, mask, Wq, Wk, Wv, Wo, norm_const):
    x = np.asarray(x)
    mask = np.asarray(mask)
    Wq = np.asarray(Wq); Wk = np.asarray(Wk); Wv = np.asarray(Wv); Wo = np.asarray(Wo)
    norm_const = np.asarray(norm_const)

    bf = ml_dtypes.bfloat16
    wkT = np.ascontiguousarray(Wk.T)
    wvT = np.ascontiguousarray(Wv.T)
    wqT = np.ascontiguousarray(Wq.T).astype(bf)
    woT = np.ascontiguousarray(Wo.T).astype(bf)
    sel = _sel_np()

    m32 = mask.astype(np.float32)
    # denom[n, h] = mask[n].sum() ** sigmoid(norm_const[h]); vscale = 1/denom
    sig = 1.0 / (1.0 + np.exp(-norm_const.astype(np.float32).reshape(H)))
    msum = m32.sum(axis=1)  # [N]
    denom = msum[:, None] ** sig[None, :]  # [N, H]
    vscale = (1.0 / denom).astype(np.float32)

    in_maps = []
    xts = {n: np.ascontiguousarray(x[n].T) for n in range(N)}
    for c in range(NCORES):
        n, j = c // 2, c % 2
        other = 1 - j
        # permuted x.T: [partner half | own half]
        xp = np.concatenate(
            [xts[n][:, other * SLOC : (other + 1) * SLOC],
             xts[n][:, j * SLOC : (j + 1) * SLOC]], axis=1
        ).astype(bf)
        mp = np.concatenate(
            [m32[n, other * SLOC : (other + 1) * SLOC],
             m32[n, j * SLOC : (j + 1) * SLOC]]
        )
        mcol = mp.reshape(NT, P).T  # [p, tt]
        mvs = np.ascontiguousarray(
            mcol[:, :, None] * vscale[n][None, None, 8 * j : 8 * j + HLOC]
        ).astype(np.float32)
        in_maps.append({
            "xt": np.ascontiguousarray(xp),
            "wk": np.ascontiguousarray(wkT[:, DH * j : DH * (j + 1)]).astype(bf),
            "wv": np.ascontiguousarray(wvT[:, DH * j : DH * (j + 1)]).astype(bf),
            "wq": wqT, "wo": woT,
            "mvs": mvs, "sel": sel,
        })
    return in_maps


def kernel(x, mask, Wq, Wk, Wv, Wo, norm_const):
    in_maps = make_in_maps(x, mask, Wq, Wk, Wv, Wo, norm_const)
    nc = build()
    res = run_bass_kernel_spmd(nc, in_maps, core_ids=list(range(NCORES)))
    out = np.empty((N, S, D), dtype=np.float32)
    for c in range(NCORES):
        n, j = c // 2, c % 2
        out[n, j * SLOC : (j + 1) * SLOC, :] = res.results[c]["out"]
    return out


# revision 27
# speedup vs baseline: 1.4056x; 1.0434x over previous
"""Trainium2 Bass kernel for nn_Attention_3633542332637 (linear/cosine attention).

Math (per batch n):
  q = x @ Wq.T ; k = x @ Wk.T ; v = x @ Wv.T          (S=4096, D=1024, H=16, HD=64)
  q,k L2-normalized per head over HD; k,v masked; v /= mask.sum()**sigmoid(nc)
  kv_h = k_h^T @ v_h  (64x64) ; attn_h = q_h @ kv_h ; out = attn @ Wo.T

Sharding: core c = 2n + j handles batch n = c//2. k/v are projected over the
FULL sequence but only for this core's 8 heads (j picks heads 8j..8j+8 via a
host-sliced half of Wk/Wv), giving a complete per-head kv with no reduction.
The pair then exchanges kv via a pairwise AllGather (64 KB), which overlaps
with the q projection. q/attn/out run over the local sequence half only (the
host permutes x.T to [partner half | own half] so the SPMD program always
treats chunks 4..7 as local).

All matmul operands are bf16 (full PE rate incl. the small-N kv matmuls,
which fp32r runs at 1/4 rate below 256 columns); accumulation is fp32 in PSUM
and all normalization statistics are fp32. q-hat stays in SBUF (bf16) — no
DRAM spill. The gathered kv is laid out as 8 block-diagonal [128,128] tiles
(one per head PAIR), so attn is 32 full-width matmuls. mask / v-denominator
fold into one [t, h] broadcast multiply on k (host-prepped); the q-head-norm
uses a block-diagonal ones matmul (sel) to reduce over partitions.
"""

import numpy as np
import ml_dtypes

import concourse.bass as bass
import concourse.mybir as mybir
import concourse.tile as tile
from concourse import bacc
from concourse.bass_utils import run_bass_kernel_spmd

N, S, D = 4, 4096, 1024
H, HD = 16, 64
P = 128
DC = D // P            # 8 contraction chunks
SLOC = S // 2          # 2048 local positions
HLOC = H // 2          # 8 heads per core
DH = HLOC * HD         # 512 = head-half feature width
NT = S // P            # 32 token tiles (full sequence)
NCHF = S // 512        # 8 full-sequence 512-token chunks
NCH = SLOC // 512      # 4 local chunks
NCORES = 8

F32 = mybir.dt.float32
F32R = mybir.dt.float32r
BF16 = mybir.dt.bfloat16
SQUARE = mybir.ActivationFunctionType.Square

REPLICA_GROUPS = [[0, 1], [2, 3], [4, 5], [6, 7]]

_BUILD_CACHE = {}


def build(reps=1, phases="both", cc=True, dbg=False, kvdt=BF16):
    key = ("nc", reps, phases, cc, dbg, str(kvdt))
    if key in _BUILD_CACHE:
        return _BUILD_CACHE[key]
    nc = bacc.Bacc("TRN2", target_bir_lowering=False, debug=False)

    # ---- I/O ----
    # x[n].T (bf16) with columns permuted to [partner half | own half]
    xt = nc.declare_dram_parameter("xt", [D, S], BF16, isOutput=False)
    # head-half slices of Wk.T / Wv.T for this core's 8 heads
    wk = nc.declare_dram_parameter("wk", [D, DH], BF16, isOutput=False)
    wv = nc.declare_dram_parameter("wv", [D, DH], BF16, isOutput=False)
    wq = nc.declare_dram_parameter("wq", [D, D], BF16, isOutput=False)
    wo = nc.declare_dram_parameter("wo", [D, D], BF16, isOutput=False)
    # mvs[p, tt, h] = mask_perm[128*tt + p] * vscale[h0 + h]  (this core's heads)
    mvs = nc.declare_dram_parameter("mvs", [P, NT, HLOC], F32, isOutput=False)
    sel = nc.declare_dram_parameter("sel", [P, P], F32, isOutput=False)  # block-diag ones
    out = nc.declare_dram_parameter("out", [SLOC, D], F32, isOutput=True)
    if dbg:
        dbg_kvblk = nc.declare_dram_parameter("dbg_kvblk", [P, DC, P], BF16, isOutput=True)
        dbg_qh = nc.declare_dram_parameter("dbg_qh", [P, DC, SLOC], BF16, isOutput=True)
        dbg_at = nc.declare_dram_parameter("dbg_at", [P, NCH, DC, 512], BF16, isOutput=True)
        dbg_khat = nc.declare_dram_parameter("dbg_khat", [P, HLOC, HD], BF16, isOutput=True)
        dbg_v = nc.declare_dram_parameter("dbg_v", [P, DH], BF16, isOutput=True)
        dbg_kvraw = nc.declare_dram_parameter("dbg_kvraw", [64, DH], BF16, isOutput=True)

    def dram3(t):
        return t.ap().rearrange("(dc p) x -> p dc x", p=P)

    with tile.TileContext(nc) as tc:
        with (
            tc.tile_pool(name="consts", bufs=1) as consts,
            tc.tile_pool(name="cdram", bufs=2, space="DRAM") as cdram,
        ):
            wk_sb = consts.tile([P, DC, DH], BF16)
            wv_sb = consts.tile([P, DC, DH], BF16)
            sel_sb = consts.tile([P, P], F32R)
            mvs_sb = consts.tile([P, NT, HLOC], F32)
            xloc = consts.tile([P, DC, SLOC], BF16)       # local x half
            wq_sb = consts.tile([P, DC, D], BF16)
            wo_sb = consts.tile([P, DC, D], BF16)
            qh_sb = consts.tile([P, DC, SLOC], BF16)      # q-hat, [e-rows, tokens]
            # block-diag kv per head pair: kvblk[:, et] = diag(kv_2et, kv_2et+1)
            kvblk = consts.tile([P, DC, P], BF16)
            at_sb = consts.tile([P, NCH, DC, 512], BF16)  # attn out, [e-rows, tokens]

            nc.sync.dma_start(out=wk_sb[:], in_=dram3(wk))
            nc.sync.dma_start(out=sel_sb[:], in_=sel.ap().bitcast(F32R))

            xt3 = dram3(xt)

            ccA_in = cdram.tile([64, DH], F32, tag="ccA_in")
            ccA_out = cdram.tile([P, DH], F32, tag="ccA_out")
            ccB_in = cdram.tile([64, DH], F32, tag="ccB_in")
            ccB_out = cdram.tile([P, DH], F32, tag="ccB_out")

            # off-diagonal blocks of kvblk stay zero across reps
            nc.vector.memset(kvblk[:].bitcast(F32), 0.0)

            def emit_phase1(first_rep=True):
              with (
                tc.tile_pool(name="pAx", bufs=2) as pAx,
                tc.tile_pool(name="pAwork", bufs=3) as pAwork,
                tc.tile_pool(name="pAstats", bufs=4) as pAstats,
                tc.tile_pool(name="pApsum", bufs=3, space="PSUM") as pApsum,
                tc.tile_pool(name="kvpool", bufs=1, space="PSUM") as kvpool,
              ):
                # two half-sequence kv accumulators so each half's exchange
                # overlaps the remaining compute (CC#1 under tiles 16-31,
                # CC#2 under the q projection)
                NTH = NT // 2
                kv_psA = kvpool.tile([64, DH], F32, tag="kvA")
                kv_psB = kvpool.tile([64, DH], F32, tag="kvB")

                def emit_kv(khat, v_sb, tt):
                    kv_ps = kv_psA if tt < NTH else kv_psB
                    t0 = tt % NTH
                    # NOTE: start=True clears has_written bits for the WHOLE
                    # bank, so only the very first matmul of the bank may set it.
                    for hh in range(HLOC):
                        nc.tensor.matmul(
                            kv_ps[:, HD * hh : HD * (hh + 1)],
                            lhsT=khat[:, hh, :],
                            rhs=v_sb[:, HD * hh : HD * (hh + 1)],
                            start=(t0 == 0 and hh == 0),
                            stop=(t0 == NTH - 1 and hh == HLOC - 1),
                            skip_group_check=True,
                        )

                def emit_exchange(kv_ps, c_in, c_out, tag):
                    kvh = pAwork.tile([64, DH], F32, tag=f"kvh{tag}")
                    nc.vector.tensor_copy(out=kvh[:], in_=kv_ps[:])
                    nc.gpsimd.dma_start(out=c_in[:], in_=kvh[:])
                    if cc:
                        nc.gpsimd.collective_compute(
                            "AllGather",
                            mybir.AluOpType.bypass,
                            replica_groups=REPLICA_GROUPS,
                            ins=[c_in.opt()],
                            outs=[c_out.opt()],
                        )
                    else:  # timing ablation: fake the gather with local copies
                        nc.gpsimd.dma_start(out=c_out[0:64, :], in_=c_in[:])
                        nc.gpsimd.dma_start(out=c_out[64:P, :], in_=c_in[:])

                pend = None
                for c in range(NCHF):
                    if c < NCH:
                        x_sb = pAx.tile([P, DC, 512], BF16)
                        if c == 0:
                            # split first chunk so tile 0 can start ASAP;
                            # interleave wv/mvs loads between the quarters
                            nc.sync.dma_start(
                                out=x_sb[:, :, 0:128], in_=xt3[:, :, 0:128]
                            )
                            if first_rep:
                                nc.sync.dma_start(out=wv_sb[:], in_=dram3(wv))
                            nc.sync.dma_start(
                                out=x_sb[:, :, 128:256], in_=xt3[:, :, 128:256]
                            )
                            if first_rep:
                                nc.sync.dma_start(out=mvs_sb[:], in_=mvs.ap())
                            nc.sync.dma_start(
                                out=x_sb[:, :, 256:512], in_=xt3[:, :, 256:512]
                            )
                        else:
                            nc.sync.dma_start(
                                out=x_sb[:], in_=xt3[:, :, 512 * c : 512 * (c + 1)]
                            )
                    elif c == NCH:
                        # local half: one resident DMA, reused in phases 2/3
                        nc.sync.dma_start(out=xloc[:], in_=xt3[:, :, SLOC:])
                    for tt4 in range(4):
                        tt = 4 * c + tt4
                        if c < NCH:
                            xs = x_sb[:, :, 128 * tt4 : 128 * (tt4 + 1)]
                        else:
                            t0 = 512 * (c - NCH) + 128 * tt4
                            xs = xloc[:, :, t0 : t0 + 128]
                        kps = pApsum.tile([P, DH], F32, tag="kps")
                        for dc in range(DC):
                            nc.tensor.matmul(
                                kps[:],
                                lhsT=xs[:, dc, :],
                                rhs=wk_sb[:, dc, :],
                                start=(dc == 0),
                                stop=(dc == DC - 1),
                            )
                        vps = pApsum.tile([P, DH], F32, tag="vps")
                        for dc in range(DC):
                            nc.tensor.matmul(
                                vps[:],
                                lhsT=xs[:, dc, :],
                                rhs=wv_sb[:, dc, :],
                                start=(dc == 0),
                                stop=(dc == DC - 1),
                            )
                        # k normalization factors (fp32 stats)
                        ksqf = pAstats.tile([P, DH], F32, tag="ksqf")
                        nc.scalar.activation(out=ksqf[:], in_=kps[:], func=SQUARE)
                        ksq = pAstats.tile([P, HLOC], F32, tag="ksq")
                        nc.vector.reduce_sum(
                            out=ksq[:],
                            in_=ksqf[:].rearrange("p (h a) -> p h a", h=HLOC),
                            axis=mybir.AxisListType.X,
                        )
                        r = pAstats.tile([P, HLOC], F32, tag="r")
                        nc.scalar.sqrt(out=r[:], in_=ksq[:])
                        nc.vector.tensor_scalar_max(out=r[:], in0=r[:], scalar1=1e-12)
                        nc.vector.reciprocal(out=r[:], in_=r[:])
                        nc.vector.tensor_mul(
                            out=r[:], in0=r[:], in1=mvs_sb[:, tt, :]
                        )
                        khat = pAwork.tile([P, HLOC, HD], kvdt, tag="khat")
                        nc.vector.tensor_tensor(
                            khat[:],
                            kps[:].rearrange("p (h a) -> p h a", h=HLOC),
                            r[:, :, None].to_broadcast((P, HLOC, HD)),
                            mybir.AluOpType.mult,
                        )
                        v_sb = pAwork.tile([P, DH], kvdt, tag="v_sb")
                        nc.scalar.copy(out=v_sb[:], in_=vps[:])
                        # software-pipelined: kv matmuls for the PREVIOUS tile,
                        # so PE never waits on this tile's stats chain
                        if pend is not None:
                            emit_kv(*pend)
                            if pend[2] == NTH - 1:
                                emit_exchange(kv_psA, ccA_in, ccA_out, "A")
                        pend = (khat, v_sb, tt)
                emit_kv(*pend)
                if dbg:
                    nc.sync.dma_start(out=dbg_khat.ap(), in_=pend[0][:])
                    nc.sync.dma_start(out=dbg_v.ap(), in_=pend[1][:])
                emit_exchange(kv_psB, ccB_in, ccB_out, "B")

                # gathered halves -> SBUF, sum, scatter into kvblk diagonals.
                # head h = 8g + 2i + two lands in kvblk[:, et=4g+i]
                gA = pAwork.tile([P, DH], F32, tag="gA")
                gB = pAwork.tile([P, DH], F32, tag="gB")
                nc.gpsimd.dma_start(out=gA[:], in_=ccA_out[:])
                nc.gpsimd.dma_start(out=gB[:], in_=ccB_out[:])
                kvsum = pAwork.tile([P, DH], BF16, tag="kvsum")
                nc.vector.tensor_tensor(
                    kvsum[:], gA[:], gB[:], mybir.AluOpType.add
                )
                if dbg:
                    nc.sync.dma_start(out=dbg_kvraw.ap(), in_=kvsum[0:64, :])
                v = kvsum[:].rearrange("p (i two e) -> p i two e", i=4, two=2)
                nc.sync.dma_start(out=kvblk[0:64, 0:4, 0:64], in_=v[0:64, :, 0, :])
                nc.sync.dma_start(out=kvblk[0:64, 4:8, 0:64], in_=v[64:P, :, 0, :])
                nc.sync.dma_start(out=kvblk[64:P, 0:4, 64:P], in_=v[0:64, :, 1, :])
                nc.sync.dma_start(out=kvblk[64:P, 4:8, 64:P], in_=v[64:P, :, 1, :])

            def emit_phase2():
              with (
                tc.tile_pool(name="pQwork", bufs=3) as pQwork,
                tc.tile_pool(name="pQpsum", bufs=3, space="PSUM") as pQpsum,
                tc.tile_pool(name="pNpsum", bufs=2, space="PSUM") as pNpsum,
              ):
                def emit_norm(qps, q2, cl, et):
                    nps = pNpsum.tile([P, 512], F32, tag="nps")
                    nc.tensor.matmul(nps[:], lhsT=sel_sb[:], rhs=q2[:],
                                     start=True, stop=True)
                    rn = pQwork.tile([P, 512], F32, tag="rn")
                    nc.scalar.sqrt(out=rn[:], in_=nps[:])
                    nc.vector.tensor_scalar_max(out=rn[:], in0=rn[:], scalar1=1e-12)
                    nc.vector.reciprocal(out=rn[:], in_=rn[:])
                    nc.vector.tensor_mul(
                        out=qh_sb[:, et, 512 * cl : 512 * (cl + 1)],
                        in0=qps[:], in1=rn[:],
                    )

                pend = None
                for cl in range(NCH):
                    for et in range(DC):
                        qps = pQpsum.tile([P, 512], F32, tag="qps")
                        for dc in range(DC):
                            nc.tensor.matmul(
                                qps[:],
                                lhsT=wq_sb[:, dc, 128 * et : 128 * (et + 1)],
                                rhs=xloc[:, dc, 512 * cl : 512 * (cl + 1)],
                                start=(dc == 0),
                                stop=(dc == DC - 1),
                            )
                        q2 = pQwork.tile([P, 512], F32R, tag="q2")
                        nc.scalar.activation(out=q2[:], in_=qps[:], func=SQUARE)
                        if pend is not None:
                            emit_norm(*pend)
                        pend = (qps, q2, cl, et)
                emit_norm(*pend)

            def emit_phase3():
              with (
                tc.tile_pool(name="pOwork", bufs=3) as pOwork,
                tc.tile_pool(name="pApsum3", bufs=3, space="PSUM") as pApsum3,
                tc.tile_pool(name="pOpsum", bufs=2, space="PSUM") as pOpsum,
              ):
                for cl in range(NCH):
                    for et in range(DC):
                        aps = pApsum3.tile([P, 512], F32, tag="aps")
                        nc.tensor.matmul(
                            aps[:],
                            lhsT=kvblk[:, et, :],
                            rhs=qh_sb[:, et, 512 * cl : 512 * (cl + 1)],
                            start=True, stop=True,
                        )
                        nc.scalar.copy(out=at_sb[:, cl, et, :], in_=aps[:])
                for cl in range(NCH):
                    for tt4 in range(4):
                        o_sb = pOwork.tile([P, D], F32, tag="o_sb")
                        for half in range(2):
                            ops = pOpsum.tile([P, 512], F32, tag="ops")
                            for ec in range(DC):
                                nc.tensor.matmul(
                                    ops[:],
                                    lhsT=at_sb[:, cl, ec, 128 * tt4 : 128 * (tt4 + 1)],
                                    rhs=wo_sb[:, ec, 512 * half : 512 * (half + 1)],
                                    start=(ec == 0),
                                    stop=(ec == DC - 1),
                                )
                            nc.scalar.copy(
                                out=o_sb[:, 512 * half : 512 * (half + 1)], in_=ops[:]
                            )
                        t0 = 512 * cl + 128 * tt4
                        nc.sync.dma_start(out=out.ap()[t0 : t0 + P, :], in_=o_sb[:])

            if phases in ("p23", "none") or reps == 0:
                nc.vector.memset(qh_sb[:].bitcast(F32), 0.0)
                nc.vector.memset(at_sb[:].bitcast(F32), 0.0)
            for _rep in range(reps):
                if phases in ("both", "p1"):
                    emit_phase1(first_rep=(_rep == 0))
                if phases in ("both", "p23"):
                    if _rep == 0:
                        # prefetch wq/wo late so they don't delay phase-1 x DMAs
                        nc.sync.dma_start(out=wq_sb[:], in_=dram3(wq))
                        nc.sync.dma_start(out=wo_sb[:], in_=dram3(wo))
                    emit_phase2()
                    emit_phase3()
            if dbg:
                nc.sync.dma_start(out=dbg_kvblk.ap(), in_=kvblk[:])
                nc.sync.dma_start(out=dbg_qh.ap(), in_=qh_sb[:])
                nc.sync.dma_start(out=dbg_at.ap(), in_=at_sb[:])

    nc.finalize()
    _BUILD_CACHE[key] = nc
    return nc


def _sel_np():
    e = np.arange(P)
    return (e[:, None] // HD == e[None, :] // HD).astype(np.float32)


def make_in_maps(x, mask, Wq, Wk, Wv, Wo, norm_const):
    x = np.asarray(x)
    mask = np.asarray(mask)
    Wq = np.asarray(Wq); Wk = np.asarray(Wk); Wv = np.asarray(Wv); Wo = np.asarray(Wo)
    norm_const = np.asarray(norm_const)

    bf = ml_dtypes.bfloat16
    wkT = np.ascontiguousarray(Wk.T)
    wvT = np.ascontiguousarray(Wv.T)
    wqT = np.ascontiguousarray(Wq.T).astype(bf)
    woT = np.ascontiguousarray(Wo.T).astype(bf)
    sel = _sel_np()

    m32 = mask.astype(np.float32)
    # denom[n, h] = mask[n].sum() ** sigmoid(norm_const[h]); vscale = 1/denom
    sig = 1.0 / (1.0 + np.exp(-norm_const.astype(np.float32).reshape(H)))
    msum = m32.sum(axis=1)  # [N]
    denom = msum[:, None] ** sig[None, :]  # [N, H]
    vscale = (1.0 / denom).astype(np.float32)

    in_maps = []
    xts = {n: np.ascontiguousarray(x[n].T) for n in range(N)}
    for c in range(NCORES):
        n, j = c // 2, c % 2
        other = 1 - j
        # permuted x.T: [partner half | own half]
        xp = np.concatenate(
            [xts[n][:, other * SLOC : (other + 1) * SLOC],
             xts[n][:, j * SLOC : (j + 1) * SLOC]], axis=1
        ).astype(bf)
        mp = np.concatenate(
            [m32[n, other * SLOC : (other + 1) * SLOC],
             m32[n, j * SLOC : (j + 1) * SLOC]]
        )
        mcol = mp.reshape(NT, P).T  # [p, tt]
        mvs = np.ascontiguousarray(
            mcol[:, :, None] * vscale[n][None, None, 8 * j : 8 * j + HLOC]
        ).astype(np.float32)
        in_maps.append({
            "xt": np.ascontiguousarray(xp),
            "wk": np.ascontiguousarray(wkT[:, DH * j : DH * (j + 1)]).astype(bf),
            "wv": np.ascontiguousarray(wvT[:, DH * j : DH * (j + 1)]).astype(bf),
            "wq": wqT, "wo": woT,
            "mvs": mvs, "sel": sel,
        })
    return in_maps


def kernel(x, mask, Wq, Wk, Wv, Wo, norm_const):
    in_maps = make_in_maps(x, mask, Wq, Wk, Wv, Wo, norm_const)
    nc = build()
    res = run_bass_kernel_spmd(nc, in_maps, core_ids=list(range(NCORES)))
    out = np.empty((N, S, D), dtype=np.float32)
    for c in range(NCORES):
        n, j = c // 2, c % 2
        out[n, j * SLOC : (j + 1) * SLOC, :] = res.results[c]["out"]
    return out
